# revision 9
# baseline (speedup 1.0000x reference)
"""Trainium2 Bass kernel for nn_Attention_72559177499201.

Reference (per batch b):
  T = q_bar[b] @ Wg + bg                  (S, H)
  scores = T @ a_bar[b].T                 (S_q, S_a)
  g = softmax(scores, axis=q)             (softmax over the QUERY axis)
  h[b] = g.T-contracted with a_bar[b]:  h[a, :] = sum_q g[q, a] * a_bar[b, q, :]

Sharding: data-parallel over batch: B=16 across 8 cores, 2 batches/core.
Forward only -> no collectives.

All matmuls and transposes run at float32r (e8m11, 1 cycle/row vs fp32's 4):
measured rel-err vs the fp32 reference is ~7e-3, within the 2e-2 gate.
Producer/consumer APs of any region consumed as f32r are both bitcast to
f32r (walrus rejects mixed-dtype producer/consumer pairs).

Per-core plan (per batch):
  stage1: T^T[k, q] = sum_h Wg[h, k] * qT[h, q]; qT via f32r PE transposes;
          two bank-aligned PSUM rounds; bias add on ACT writes T^T to SBUF.
  stage2: S_T[a, q] = aT_chunk^T @ T^T per 128-key a-tile so the softmax
          axis q lands on the free axis.
  softmax along the free axis of S_T: per-bank maxes + combine (DVE), one
          exp with bias=-max and accumulated sum (ACT), reciprocal (DVE).
  stage3: g transposed back to [q, a] via f32r PE transposes, then
          h[a, :] = sum_q g[q, a] * a_bar[q, :], scaled by 1/Z on the
          PSUM->SBUF copy (ACT), DMA out.

Engine/queue scheduling (the fp32->f32r switch makes PE ~3x faster, so the
softmax/copy side-chains must be kept off the PE critical path):
  - a_bar is DMA'd once per batch into a_r (natural layout) on the ACT DGE
    queue at batch start; both the per-tile transposes (stage 2) and the
    stage-3 rhs read it. q loads ride the SP DGE queue (double-buffered
    qnat) so neither queue head-of-line blocks the other. Output DMAs go on
    the ACT queue.
  - Wg is loaded in 8 per-chunk DMAs so stage-1's first matmul only waits
    for chunk 0.
  - Per a-tile emission: front(i) [aT transposes + scores], maxes(i) [DVE,
    ahead of back's g-copies in the DVE FIFO so exp isn't delayed], g
    transposes(i-1) with copies alternating ACT/DVE, exp(i) [ACT, ahead of
    h-mul], stage3(i-1). This keeps the exp(i) done well before scores(i+1)
    needs the PSUM scores region back (WAR), and g_r copies ready just as
    stage3 consumes them.
"""
import os
import sys

sys.path.insert(0, "/opt/trn_rl_repo")

from contextlib import ExitStack

import numpy as np

B, S, H = 16, 2048, 1024
NCORES = 8
BPC = B // NCORES  # 2 batches per core

_cache = {}


def _build():
    import concourse.tile as tile
    from concourse import bacc, mybir
    from concourse.masks import make_identity

    F32 = mybir.dt.float32
    F32R = mybir.dt.float32r

    KC = H // 128  # 8 contraction chunks
    Q1 = 256       # stage-1 q chunk width
    AT = S // 128  # 16 a-tiles
    HC2 = H // 512  # 2 output h chunks

    nc = bacc.Bacc("TRN2", target_bir_lowering=False, debug=False,
                   num_devices=NCORES)
    q_d = nc.declare_dram_parameter("q_bar", [BPC, S, H], F32, isOutput=False)
    a_d = nc.declare_dram_parameter("a_bar", [BPC, S, H], F32, isOutput=False)
    wg_d = nc.declare_dram_parameter("Wg", [H, H], F32, isOutput=False)
    bg_d = nc.declare_dram_parameter("bg", [H], F32, isOutput=False)
    out_d = nc.declare_dram_parameter("out", [BPC, S, H], F32, isOutput=True)

    with tile.TileContext(nc) as tc, ExitStack() as ctx:
        const = ctx.enter_context(tc.tile_pool(name="const", bufs=1))
        big = ctx.enter_context(tc.tile_pool(name="big", bufs=1))
        st1 = ctx.enter_context(tc.tile_pool(name="st1", bufs=1))
        qbuf = ctx.enter_context(tc.tile_pool(name="qbuf", bufs=2))
        st2 = ctx.enter_context(tc.tile_pool(name="st2", bufs=2))
        st_ps = ctx.enter_context(tc.tile_pool(name="st_ps", bufs=1, space="PSUM"))
        tr_ps = ctx.enter_context(tc.tile_pool(name="tr_ps", bufs=2, space="PSUM"))
        h_ps = ctx.enter_context(tc.tile_pool(name="h_ps", bufs=1, space="PSUM"))

        cb = const.tile([128, 136], F32, tag="identbg")
        ident = cb[:, 0:128]
        make_identity(nc, ident)
        bg_sb = cb[:, 128:136]                           # bg[k] at [k%128, 128+k//128]
        nc.sync.dma_start(bg_sb, bg_d.rearrange("(ko p) -> p ko", p=128))
        # f32r identity in its OWN tile (the BIR verifier checks f32r-rounded
        # producers per TENSOR, so it must not share a tile with bg/ident),
        # produced by a DVE cast copy.
        identr_t = const.tile([128, 128], F32, tag="identr")
        identr = identr_t[:].bitcast(F32R)
        nc.vector.tensor_copy(identr, ident)
        wg_sb = const.tile([128, KC, H], F32, tag="wg")  # [h_in_chunk, hc, k]
        wg_src = wg_d.rearrange("(ho p) k -> p ho k", p=128)
        for hc in range(KC):  # chunked so stage-1 only waits on chunk 0
            nc.scalar.dma_start(wg_sb[:, hc, :].bitcast(F32R),
                                wg_src[:, hc, :].bitcast(F32R))

        for b in range(BPC):
            # T^T: [k within chunk, kc, q]  (f32r-produced)
            T_sb = big.tile([128, KC, S], F32, tag="T")
            # a_bar natural: [q within chunk, sc, h]; loaded ONCE per batch,
            # read by the stage-2 aT transposes and as the stage-3 rhs.
            a_r = big.tile([128, AT, H], F32, tag="ar")
            for sc in range(AT):
                nc.scalar.dma_start(
                    a_r[:, sc, :].bitcast(F32R),
                    a_d[b, sc * 128:(sc + 1) * 128, :].bitcast(F32R),
                )

            # ---- stage 1: T^T = Wg^T-contraction with q^T ----
            for qc in range(S // Q1):  # 8 chunks of 256 q
                qT = st1.tile([128, KC, Q1], F32, tag="qT")
                for qsc in range(Q1 // 128):
                    qnat = qbuf.tile([128, H], F32, tag="ld1024")
                    row0 = qc * Q1 + qsc * 128
                    nc.sync.dma_start(qnat[:].bitcast(F32R),
                                      q_d[b, row0:row0 + 128, :].bitcast(F32R))
                    for hg in range(2):  # two groups of 4 transposes per bank
                        pt = tr_ps.tile([128, 512], F32, tag="tr")
                        for j in range(4):
                            hc = hg * 4 + j
                            nc.tensor.transpose(
                                pt[:, j * 128:(j + 1) * 128].bitcast(F32R),
                                qnat[:, hc * 128:(hc + 1) * 128].bitcast(F32R),
                                identr,
                            )
                        nc.vector.tensor_copy(
                            qT[:, hg * 4:(hg + 1) * 4,
                               qsc * 128:qsc * 128 + 128].bitcast(F32R),
                            pt[:].bitcast(F32R).rearrange("p (j q) -> p j q", j=4),
                        )
                # one 256-wide accumulation group per 512-elem PSUM bank:
                # start=True clears has_written for the WHOLE bank, so groups
                # must not share banks.
                st = st_ps.tile([128, 2048], F32, tag="st")
                for rnd in range(2):
                    for hc in range(KC):
                        for kg in range(4):
                            kc = rnd * 4 + kg
                            nc.tensor.matmul(
                                st[:, kg * 512:kg * 512 + Q1],
                                wg_sb[:, hc, kc * 128:(kc + 1) * 128].bitcast(F32R),
                                qT[:, hc, :].bitcast(F32R),
                                start=(hc == 0),
                                stop=(hc == KC - 1),
                            )
                    for kg in range(4):
                        kc = rnd * 4 + kg
                        nc.scalar.add(
                            T_sb[:, kc, qc * Q1:(qc + 1) * Q1].bitcast(F32R),
                            st[:, kg * 512:kg * 512 + Q1],
                            bg_sb[:, kc:kc + 1],
                        )

            # ---- stage 2 + softmax + stage 3, staggered by one a-tile ----
            state = {}

            def emit_front(i):
                # aT: transpose this a-tile's rows out of a_r
                aT = st1.tile([128, KC, 128], F32, tag="qT")
                for hg in range(2):
                    pt = tr_ps.tile([128, 512], F32, tag="tr")
                    for j in range(4):
                        hc = hg * 4 + j
                        nc.tensor.transpose(
                            pt[:, j * 128:(j + 1) * 128].bitcast(F32R),
                            a_r[:, i, hc * 128:(hc + 1) * 128].bitcast(F32R),
                            identr,
                        )
                    nc.scalar.copy(
                        aT[:, hg * 4:(hg + 1) * 4, :].bitcast(F32R),
                        pt[:].bitcast(F32R).rearrange("p (j q) -> p j q", j=4),
                    )
                # scores S_T[a, q] for this a-tile
                stt = st_ps.tile([128, 2048], F32, tag="st")
                for kc in range(KC):
                    for qcc in range(S // 512):
                        nc.tensor.matmul(
                            stt[:, qcc * 512:(qcc + 1) * 512],
                            aT[:, kc, :].bitcast(F32R),
                            T_sb[:, kc, qcc * 512:(qcc + 1) * 512].bitcast(F32R),
                            start=(kc == 0),
                            stop=(kc == KC - 1),
                        )
                state[i] = stt

            def emit_max(i):
                stt = state[i]
                stat = st2.tile([128, 8], F32, tag="stats")
                for qm in range(4):
                    nc.vector.tensor_reduce(
                        stat[:, 4 + qm:5 + qm], stt[:, qm * 512:(qm + 1) * 512],
                        axis=mybir.AxisListType.X, op=mybir.AluOpType.max,
                    )
                nc.vector.tensor_reduce(
                    stat[:, 0:1], stat[:, 4:8], axis=mybir.AxisListType.X,
                    op=mybir.AluOpType.max, negate=True,
                )
                state[(i, "stat")] = stat

            def emit_exp(i):
                stt = state.pop(i)
                stat = state[(i, "stat")]
                gT = st1.tile([128, S], F32, tag="gT")
                nc.scalar.activation(
                    gT[:].bitcast(F32R), stt[:], mybir.ActivationFunctionType.Exp,
                    bias=stat[:, 0:1], scale=1.0, accum_out=stat[:, 1:2],
                )
                nc.vector.reciprocal(stat[:, 2:3], stat[:, 1:2])
                state[(i, "g")] = gT

            def emit_back_tr(i):
                gT = state.pop((i, "g"))
                g_r = st1.tile([128, AT, 128], F32R, tag="gr")
                for qg in range(4):  # 16 transposes, batched 4 per bank
                    pt = tr_ps.tile([128, 512], F32, tag="tr")
                    for j in range(4):
                        qc = qg * 4 + j
                        nc.tensor.transpose(
                            pt[:, j * 128:(j + 1) * 128].bitcast(F32R),
                            gT[:, qc * 128:(qc + 1) * 128].bitcast(F32R),
                            identr,
                        )
                    # alternate copy engines: ACT takes qg0/qg2, DVE qg1/qg3,
                    # so neither queue serializes stage-3's operand feed.
                    dst = g_r[:, qg * 4:(qg + 1) * 4, :]
                    src = pt[:].bitcast(F32R).rearrange("p (j q) -> p j q", j=4)
                    if qg % 2 == 0:
                        nc.scalar.copy(dst, src)
                    else:
                        nc.vector.tensor_copy(dst, src)
                state[(i, "gr")] = g_r

            def emit_back_mm(i):
                g_r = state.pop((i, "gr"))
                stat = state.pop((i, "stat"))
                hp = h_ps.tile([128, H], F32, tag="hp")
                for hc2 in range(HC2):
                    for qq in range(AT):
                        nc.tensor.matmul(
                            hp[:, hc2 * 512:(hc2 + 1) * 512],
                            g_r[:, qq, :],
                            a_r[:, qq, hc2 * 512:(hc2 + 1) * 512].bitcast(F32R),
                            start=(qq == 0),
                            stop=(qq == AT - 1),
                        )
                h_sb = st1.tile([128, H], F32, tag="h")
                nc.scalar.mul(h_sb[:], hp[:], stat[:, 2:3])
                nc.scalar.dma_start(out_d[b, i * 128:(i + 1) * 128, :], h_sb[:])

            prev = None
            for i in range(AT + 1):
                if i < AT:
                    emit_front(i)
                    emit_max(i)
                if prev is not None:
                    emit_back_tr(prev)
                if i < AT:
                    emit_exp(i)
                if prev is not None:
                    emit_back_mm(prev)
                prev = i if i < AT else None

    nc.compile()
    return nc


def _get_nc():
    if "nc" not in _cache:
        _cache["nc"] = _build()
    return _cache["nc"]


def _run(q_bar, a_bar, Wg, bg, trace=False):
    from concourse.bass_utils import run_bass_kernel_spmd

    q_bar = np.ascontiguousarray(q_bar, dtype=np.float32)
    a_bar = np.ascontiguousarray(a_bar, dtype=np.float32)
    Wg = np.ascontiguousarray(Wg, dtype=np.float32)
    bg = np.ascontiguousarray(bg, dtype=np.float32)

    nc = _get_nc()
    in_maps = []
    for c in range(NCORES):
        in_maps.append({
            "q_bar": q_bar[c * BPC:(c + 1) * BPC],
            "a_bar": a_bar[c * BPC:(c + 1) * BPC],
            "Wg": Wg,
            "bg": bg,
        })
    res = run_bass_kernel_spmd(nc, in_maps, list(range(NCORES)), trace=trace)
    out = np.concatenate([res.results[c]["out"] for c in range(NCORES)], axis=0)
    return out, res


def kernel(q_bar, a_bar, Wg, bg):
    out, _ = _run(q_bar, a_bar, Wg, bg, trace=False)
    return out


# revision 13
# speedup vs baseline: 1.1197x; 1.1197x over previous
"""Trainium2 Bass kernel for nn_Attention_72559177499201.

Reference (per batch b):
  T = q_bar[b] @ Wg + bg                  (S, H)
  scores = T @ a_bar[b].T                 (S_q, S_a)
  g = softmax(scores, axis=q)             (softmax over the QUERY axis)
  h[b] = g.T-contracted with a_bar[b]:  h[a, :] = sum_q g[q, a] * a_bar[b, q, :]

Sharding: data-parallel over batch: B=16 across 8 cores, 2 batches/core.
Forward only -> no collectives.

All matmuls and transposes run at float32r (e8m11, 1 cycle/row vs fp32's 4):
measured rel-err vs the fp32 reference is ~7e-3, within the 2e-2 gate.
Producer/consumer APs of any region consumed as f32r are both bitcast to
f32r (walrus rejects mixed-dtype producer/consumer pairs).

Per-core plan (per batch):
  stage1: T^T[k, q] = sum_h Wg[h, k] * qT[h, q]; qT via f32r PE transposes;
          two bank-aligned PSUM rounds; bias add on ACT writes T^T to SBUF.
  stage2: S_T[a, q] = aT_chunk^T @ T^T per 128-key a-tile so the softmax
          axis q lands on the free axis.
  softmax along the free axis of S_T: per-bank maxes + combine (DVE), one
          exp with bias=-max and accumulated sum (ACT), reciprocal (DVE).
  stage3: g transposed back to [q, a] via f32r PE transposes, then
          h[a, :] = sum_q g[q, a] * a_bar[q, :], scaled by 1/Z on the
          PSUM->SBUF copy (ACT), DMA out.

Engine/queue scheduling (the fp32->f32r switch makes PE ~3x faster, so the
softmax/copy side-chains must be kept off the PE critical path):
  - a_bar is DMA'd once per batch into a_r (natural layout) on the ACT DGE
    queue at batch start; both the per-tile transposes (stage 2) and the
    stage-3 rhs read it. q loads ride the SP DGE queue (double-buffered
    qnat) so neither queue head-of-line blocks the other. Output DMAs go on
    the ACT queue.
  - Wg is loaded in 8 per-chunk DMAs so stage-1's first matmul only waits
    for chunk 0.
  - Per a-tile emission: front(i) [aT transposes + scores], maxes(i) [DVE,
    ahead of back's g-copies in the DVE FIFO so exp isn't delayed], g
    transposes(i-1) with copies alternating ACT/DVE, exp(i) [ACT, ahead of
    h-mul], stage3(i-1). This keeps the exp(i) done well before scores(i+1)
    needs the PSUM scores region back (WAR), and g_r copies ready just as
    stage3 consumes them.
"""
import os
import sys

sys.path.insert(0, "/opt/trn_rl_repo")

from contextlib import ExitStack

import numpy as np

B, S, H = 16, 2048, 1024
NCORES = 8
BPC = B // NCORES  # 2 batches per core

_cache = {}


def _build():
    import concourse.tile as tile
    from concourse import bacc, mybir
    from concourse.masks import make_identity

    F32 = mybir.dt.float32
    F32R = mybir.dt.float32r

    KC = H // 128  # 8 contraction chunks
    Q1 = 256       # stage-1 q chunk width
    AT = S // 128  # 16 a-tiles
    HC2 = H // 512  # 2 output h chunks

    nc = bacc.Bacc("TRN2", target_bir_lowering=False, debug=False,
                   num_devices=NCORES)
    q_d = nc.declare_dram_parameter("q_bar", [BPC, S, H], F32, isOutput=False)
    a_d = nc.declare_dram_parameter("a_bar", [BPC, S, H], F32, isOutput=False)
    wg_d = nc.declare_dram_parameter("Wg", [H, H], F32, isOutput=False)
    bg_d = nc.declare_dram_parameter("bg", [H], F32, isOutput=False)
    out_d = nc.declare_dram_parameter("out", [BPC, S, H], F32, isOutput=True)

    with tile.TileContext(nc) as tc, ExitStack() as ctx:
        const = ctx.enter_context(tc.tile_pool(name="const", bufs=1))
        big = ctx.enter_context(tc.tile_pool(name="big", bufs=1))
        st1 = ctx.enter_context(tc.tile_pool(name="st1", bufs=1))
        qbuf = ctx.enter_context(tc.tile_pool(name="qbuf", bufs=2))
        st2 = ctx.enter_context(tc.tile_pool(name="st2", bufs=2))
        st_ps = ctx.enter_context(tc.tile_pool(name="st_ps", bufs=1, space="PSUM"))
        tr_ps = ctx.enter_context(tc.tile_pool(name="tr_ps", bufs=2, space="PSUM"))
        h_ps = ctx.enter_context(tc.tile_pool(name="h_ps", bufs=1, space="PSUM"))

        cb = const.tile([128, 136], F32, tag="identbg")
        ident = cb[:, 0:128]
        make_identity(nc, ident)
        bg_sb = cb[:, 128:136]                           # bg[k] at [k%128, 128+k//128]
        nc.sync.dma_start(bg_sb, bg_d.rearrange("(ko p) -> p ko", p=128))
        # f32r identity in its OWN tile (the BIR verifier checks f32r-rounded
        # producers per TENSOR, so it must not share a tile with bg/ident),
        # produced by a DVE cast copy.
        identr_t = const.tile([128, 128], F32, tag="identr")
        identr = identr_t[:].bitcast(F32R)
        nc.vector.tensor_copy(identr, ident)
        wg_sb = const.tile([128, KC, H], F32, tag="wg")  # [h_in_chunk, hc, k]
        wg_src = wg_d.rearrange("(ho p) k -> p ho k", p=128)
        for hc in range(KC):  # chunked so stage-1 only waits on chunk 0
            nc.scalar.dma_start(wg_sb[:, hc, :].bitcast(F32R),
                                wg_src[:, hc, :].bitcast(F32R))

        for b in range(BPC):
            # T^T: [k within chunk, kc, q]  (f32r-produced)
            T_sb = big.tile([128, KC, S], F32, tag="T")
            # a_bar natural: [q within chunk, sc, h]; loaded ONCE per batch,
            # read by the stage-2 aT transposes and as the stage-3 rhs.
            a_r = big.tile([128, AT, H], F32, tag="ar")

            def emit_a_fill(sc):
                nc.scalar.dma_start(
                    a_r[:, sc, :].bitcast(F32R),
                    a_d[b, sc * 128:(sc + 1) * 128, :].bitcast(F32R),
                )

            # ---- stage 1: T^T = Wg^T-contraction with q^T ----
            # a fills are spread 2-per-qc-chunk so the ACT DGE queue trickles
            # them out between the q loads instead of hogging HBM bandwidth.
            for qc in range(S // Q1):  # 8 chunks of 256 q
                qT = st1.tile([128, KC, Q1], F32, tag="qT")
                for qsc in range(Q1 // 128):
                    qnat = qbuf.tile([128, H], F32, tag="ld1024")
                    row0 = qc * Q1 + qsc * 128
                    nc.sync.dma_start(qnat[:].bitcast(F32R),
                                      q_d[b, row0:row0 + 128, :].bitcast(F32R))
                    for hg in range(2):  # two groups of 4 transposes per bank
                        pt = tr_ps.tile([128, 512], F32, tag="tr")
                        for j in range(4):
                            hc = hg * 4 + j
                            nc.tensor.transpose(
                                pt[:, j * 128:(j + 1) * 128].bitcast(F32R),
                                qnat[:, hc * 128:(hc + 1) * 128].bitcast(F32R),
                                identr,
                            )
                        nc.vector.tensor_copy(
                            qT[:, hg * 4:(hg + 1) * 4,
                               qsc * 128:qsc * 128 + 128].bitcast(F32R),
                            pt[:].bitcast(F32R).rearrange("p (j q) -> p j q", j=4),
                        )
                # one 256-wide accumulation group per 512-elem PSUM bank:
                # start=True clears has_written for the WHOLE bank, so groups
                # must not share banks.
                st = st_ps.tile([128, 2048], F32, tag="st")
                for rnd in range(2):
                    for hc in range(KC):
                        for kg in range(4):
                            kc = rnd * 4 + kg
                            nc.tensor.matmul(
                                st[:, kg * 512:kg * 512 + Q1],
                                wg_sb[:, hc, kc * 128:(kc + 1) * 128].bitcast(F32R),
                                qT[:, hc, :].bitcast(F32R),
                                start=(hc == 0),
                                stop=(hc == KC - 1),
                            )
                    for kg in range(4):
                        kc = rnd * 4 + kg
                        nc.scalar.add(
                            T_sb[:, kc, qc * Q1:(qc + 1) * Q1].bitcast(F32R),
                            st[:, kg * 512:kg * 512 + Q1],
                            bg_sb[:, kc:kc + 1],
                        )
                emit_a_fill(2 * qc)
                emit_a_fill(2 * qc + 1)

            # ---- stage 2 + softmax + stage 3, staggered by one a-tile ----
            state = {}

            def emit_front_tr(i):
                # aT: transpose this a-tile's rows out of a_r.  Emitted one
                # tile EARLY (before stage3(i-2)) so the PSUM->SBUF copies
                # complete long before scores(i) consumes aT.
                aT = st1.tile([128, KC, 128], F32, tag="qT")
                for hg in range(2):
                    pt = tr_ps.tile([128, 512], F32, tag="tr")
                    for j in range(4):
                        hc = hg * 4 + j
                        nc.tensor.transpose(
                            pt[:, j * 128:(j + 1) * 128].bitcast(F32R),
                            a_r[:, i, hc * 128:(hc + 1) * 128].bitcast(F32R),
                            identr,
                        )
                    nc.scalar.copy(
                        aT[:, hg * 4:(hg + 1) * 4, :].bitcast(F32R),
                        pt[:].bitcast(F32R).rearrange("p (j q) -> p j q", j=4),
                    )
                state[(i, "aT")] = aT

            def emit_front_mm(i):
                aT = state.pop((i, "aT"))
                # scores S_T[a, q] for this a-tile
                stt = st_ps.tile([128, 2048], F32, tag="st")
                for kc in range(KC):
                    for qcc in range(S // 512):
                        nc.tensor.matmul(
                            stt[:, qcc * 512:(qcc + 1) * 512],
                            aT[:, kc, :].bitcast(F32R),
                            T_sb[:, kc, qcc * 512:(qcc + 1) * 512].bitcast(F32R),
                            start=(kc == 0),
                            stop=(kc == KC - 1),
                        )
                state[i] = stt

            def emit_max(i):
                stt = state[i]
                stat = st2.tile([128, 8], F32, tag="stats")
                for qm in range(4):
                    nc.vector.tensor_reduce(
                        stat[:, 4 + qm:5 + qm], stt[:, qm * 512:(qm + 1) * 512],
                        axis=mybir.AxisListType.X, op=mybir.AluOpType.max,
                    )
                nc.vector.tensor_reduce(
                    stat[:, 0:1], stat[:, 4:8], axis=mybir.AxisListType.X,
                    op=mybir.AluOpType.max, negate=True,
                )
                state[(i, "stat")] = stat

            def emit_exp(i):
                stt = state.pop(i)
                stat = state[(i, "stat")]
                gT = st1.tile([128, S], F32, tag="gT")
                nc.scalar.activation(
                    gT[:].bitcast(F32R), stt[:], mybir.ActivationFunctionType.Exp,
                    bias=stat[:, 0:1], scale=1.0, accum_out=stat[:, 1:2],
                )
                nc.vector.reciprocal(stat[:, 2:3], stat[:, 1:2])
                state[(i, "g")] = gT

            def emit_back_tr(i):
                gT = state.pop((i, "g"))
                g_r = st1.tile([128, AT, 128], F32R, tag="gr")
                for qg in range(4):  # 16 transposes, batched 4 per bank
                    pt = tr_ps.tile([128, 512], F32, tag="tr")
                    for j in range(4):
                        qc = qg * 4 + j
                        nc.tensor.transpose(
                            pt[:, j * 128:(j + 1) * 128].bitcast(F32R),
                            gT[:, qc * 128:(qc + 1) * 128].bitcast(F32R),
                            identr,
                        )
                    # alternate copy engines: ACT takes qg0/qg2, DVE qg1/qg3,
                    # so neither queue serializes stage-3's operand feed.
                    dst = g_r[:, qg * 4:(qg + 1) * 4, :]
                    src = pt[:].bitcast(F32R).rearrange("p (j q) -> p j q", j=4)
                    if qg % 2 == 0:
                        nc.scalar.copy(dst, src)
                    else:
                        nc.vector.tensor_copy(dst, src)
                state[(i, "gr")] = g_r

            def emit_back_mm(i):
                g_r = state.pop((i, "gr"))
                stat = state.pop((i, "stat"))
                hp = h_ps.tile([128, H], F32, tag="hp")
                for hc2 in range(HC2):
                    for qq in range(AT):
                        nc.tensor.matmul(
                            hp[:, hc2 * 512:(hc2 + 1) * 512],
                            g_r[:, qq, :],
                            a_r[:, qq, hc2 * 512:(hc2 + 1) * 512].bitcast(F32R),
                            start=(qq == 0),
                            stop=(qq == AT - 1),
                        )
                h_sb = st1.tile([128, H], F32, tag="h")
                nc.scalar.mul(h_sb[:], hp[:], stat[:, 2:3])
                nc.scalar.dma_start(out_d[b, i * 128:(i + 1) * 128, :], h_sb[:])

            # PE order per period: scores(i), g-transposes(i-1),
            # aT-transposes(i+1), stage3(i-1) — so every PSUM->SBUF copy has
            # a long PE block between its producer and its consumer.
            emit_front_tr(0)
            prev = None
            for i in range(AT + 1):
                if i < AT:
                    emit_front_mm(i)
                    emit_max(i)
                if prev is not None:
                    emit_back_tr(prev)
                if i + 1 < AT:
                    emit_front_tr(i + 1)
                if i < AT:
                    emit_exp(i)
                if prev is not None:
                    emit_back_mm(prev)
                prev = i if i < AT else None

    nc.compile()
    return nc


def _get_nc():
    if "nc" not in _cache:
        _cache["nc"] = _build()
    return _cache["nc"]


def _run(q_bar, a_bar, Wg, bg, trace=False):
    from concourse.bass_utils import run_bass_kernel_spmd

    q_bar = np.ascontiguousarray(q_bar, dtype=np.float32)
    a_bar = np.ascontiguousarray(a_bar, dtype=np.float32)
    Wg = np.ascontiguousarray(Wg, dtype=np.float32)
    bg = np.ascontiguousarray(bg, dtype=np.float32)

    nc = _get_nc()
    in_maps = []
    for c in range(NCORES):
        in_maps.append({
            "q_bar": q_bar[c * BPC:(c + 1) * BPC],
            "a_bar": a_bar[c * BPC:(c + 1) * BPC],
            "Wg": Wg,
            "bg": bg,
        })
    res = run_bass_kernel_spmd(nc, in_maps, list(range(NCORES)), trace=trace)
    out = np.concatenate([res.results[c]["out"] for c in range(NCORES)], axis=0)
    return out, res


def kernel(q_bar, a_bar, Wg, bg):
    out, _ = _run(q_bar, a_bar, Wg, bg, trace=False)
    return out
